# revision 22
# baseline (speedup 1.0000x reference)
"""Trainium2 Bass kernel for nn_LinkPredictor (2-layer GCN + edge-dot decode).

Strategy (8 NeuronCores, SPMD), v2 architecture:
  - Nodes sharded: core c owns rows [c*12544, (c+1)*12544).
  - Scale folding: table rows are pre-scaled by dinv[node]; the remaining
    dinv[dst] factor is applied per-window after aggregation (it commutes
    with the right-multiplication by W).  GCN conv = relu(dinv*(agg @ W)+b),
    agg[m] = self_row[m] + sum_e table[src[e]].
  - Layer-1 table (dinv*x, bf16) is host-replicated to every core: no
    AllGather needed before layer 1.  Only 2 AllGathers total (table1, tablez).
  - Edges grouped by (dst window of 128, src chunk of 25088); slots padded
    to 128-multiples per group.  Slot -> dst position is applied via one-hot
    S tiles built on DVE with a single is_equal op (pads get dstloc=-1 so
    their S row is zero; pad gathers read row 0, finite garbage * 0 = 0).
  - Aggregation: PSUM accumulation of S_t^T @ Mt_t (PE), self term via
    identity-weights matmul of the core's own contiguous table rows.
  - Per window: transpose agg (PE), matmul aggT @ W (PE), fused
    (ps*dinv[m])+b (DVE scalar_tensor_tensor), relu (Scalar engine).
  - Decode: label edge j assigned to the core owning src; s-side gathered
    from the core's own shardz (overlaps the last AllGather), d-side from
    the all-gathered tablez; dot = tensor_tensor mult + reduce.
"""
import contextlib
import math
import numpy as np
import ml_dtypes

import concourse.bass as bass
import concourse.tile as tile
from concourse import bacc, mybir
from concourse.bass_utils import run_bass_kernel_spmd
from concourse.tile_rust import add_dep_helper

F32 = mybir.dt.float32
BF16 = mybir.dt.bfloat16
I16 = mybir.dt.int16
BF = ml_dtypes.bfloat16


class Cfg:
    def __init__(self, N=100000, E=1600000, EL=100000, D=128, ncores=8,
                 nw=98, nchunks=4, wb=7):
        self.N, self.E, self.EL, self.D, self.NC = N, E, EL, D, ncores
        self.NW = nw                      # windows (128 nodes each) per core
        self.SHARD = nw * 128             # real nodes per core (12544)
        self.SROWS = self.SHARD + 128     # shard rows incl zero tail (12672)
        self.NP = self.SHARD * ncores     # real node count padded (100352)
        self.TROWS = self.SROWS * ncores  # table rows (101376)
        assert self.NP >= N
        self.NCH = nchunks                # src chunks (int16 index range)
        self.CHROWS = self.TROWS // nchunks  # 25344
        assert self.CHROWS <= 32767
        self.ZLOC = self.SHARD            # chunk-local zero row (12544)
        self.WB = wb                      # windows per gather batch
        assert nw % wb == 0
        self.NBATCH = nw // wb


DEFAULT = Cfg()


def _wrap_idxs(idx):
    """[n] ints -> [128, n//16] int16 wrapped in 16 partitions, replicated 8x."""
    n = len(idx)
    assert n % 16 == 0
    w = np.asarray(idx, dtype=np.int16).reshape(n // 16, 16).T
    return np.ascontiguousarray(np.tile(w, (8, 1)))


def host_prep(cfg, x, edge_index, edge_label_index, W1, b1, W2, b2):
    c = cfg
    src = np.asarray(edge_index[0], dtype=np.int64)
    dst = np.asarray(edge_index[1], dtype=np.int64)
    deg = np.bincount(dst, minlength=c.N).astype(np.float64) + 1.0
    dinv = 1.0 / np.sqrt(deg)                       # [N]
    dinv_p = np.zeros(c.NP, dtype=np.float32)
    dinv_p[:c.N] = dinv.astype(np.float32)

    # ---- per-core node permutation: sort by degree (desc) -----------------
    deg_p = np.zeros(c.NP, dtype=np.int64)
    deg_p[:c.N] = (deg - 1).astype(np.int64)
    perm_pos = np.empty(c.NP, dtype=np.int64)       # local node r -> slot p
    for core in range(c.NC):
        d_loc = deg_p[core * c.SHARD:(core + 1) * c.SHARD]
        o = np.argsort(-d_loc, kind="stable")
        pp = np.empty(c.SHARD, dtype=np.int64)
        pp[o] = np.arange(c.SHARD)
        perm_pos[core * c.SHARD:(core + 1) * c.SHARD] = pp
    # global node g -> table row
    g_all = np.arange(c.NP, dtype=np.int64)
    row_of = (g_all // c.SHARD) * c.SROWS + perm_pos

    # ---- edge slot structure ---------------------------------------------
    core_of = dst // c.SHARD
    p_of = perm_pos[dst]                            # dst slot within core
    w_of = p_of // 128
    m_of = p_of % 128
    srow = row_of[src]
    ch_of = srow // c.CHROWS
    loc_of = srow % c.CHROWS

    # rank of edge within (core, w, ch, m)
    key1 = (((core_of * c.NW + w_of) * c.NCH + ch_of) * 128 + m_of)
    order = np.argsort(key1, kind="stable")
    cnt1 = np.bincount(key1, minlength=c.NC * c.NW * c.NCH * 128)
    st1 = np.zeros(len(cnt1) + 1, dtype=np.int64)
    np.cumsum(cnt1, out=st1[1:])
    rank = np.empty(c.E, dtype=np.int64)
    rank[order] = np.arange(c.E) - st1[key1[order]]

    # choose K (structured depth) and SB (spill blocks) per (w, ch)
    cnts = cnt1.reshape(c.NC, c.NW, c.NCH, 128)
    KMAX = 24
    best_cost = None
    Kwc = np.zeros((c.NW, c.NCH), dtype=np.int64)
    SBwc = np.zeros((c.NW, c.NCH), dtype=np.int64)
    for K in range(KMAX + 1):
        spill = np.maximum(cnts - K, 0).sum(axis=3)          # [NC, NW, NCH]
        SB = np.ceil(spill / 128).astype(np.int64).max(axis=0)  # [NW, NCH]
        cost = K + SB + 0.01 * SB
        if best_cost is None:
            best_cost = cost.astype(np.float64)
            Kwc[:] = K
            SBwc[:] = SB
        else:
            better = cost < best_cost
            best_cost = np.where(better, cost, best_cost)
            Kwc = np.where(better, K, Kwc)
            SBwc = np.where(better, SB, SBwc)

    # cell layout: batch -> chunk -> window; cell = K*128 + SB*128 slots
    cell_slot0 = np.zeros((c.NW, c.NCH), dtype=np.int64)
    spill_col0 = np.zeros((c.NW, c.NCH), dtype=np.int64)
    goff = {}
    pos = 0
    spill_blocks = 0
    for b in range(c.NBATCH):
        for ch in range(c.NCH):
            run0 = pos
            for w in range(b * c.WB, (b + 1) * c.WB):
                cell_slot0[w, ch] = pos
                spill_col0[w, ch] = spill_blocks
                pos += (Kwc[w, ch] + SBwc[w, ch]) * 128
                spill_blocks += SBwc[w, ch]
            goff[(b, ch)] = (run0, pos - run0)
    TOT = pos
    NBLK = TOT // 128
    NSPILL = max(spill_blocks, 1)

    # default gidx (pads): structured -> zero row, spill -> row 0
    gdefault = np.zeros(TOT, dtype=np.int64)
    for w in range(c.NW):
        for ch in range(c.NCH):
            s0 = cell_slot0[w, ch]
            gdefault[s0:s0 + Kwc[w, ch] * 128] = c.ZLOC

    # per-edge slot
    K_e = Kwc[w_of, ch_of]
    is_struct = rank < K_e
    slot = np.empty(c.E, dtype=np.int64)
    slot[is_struct] = (cell_slot0[w_of[is_struct], ch_of[is_struct]]
                       + rank[is_struct] * 128 + m_of[is_struct])
    # spill rank within (core, w, ch)
    sp = ~is_struct
    key2 = (core_of * c.NW + w_of) * c.NCH + ch_of
    k2s = key2[sp]
    o2 = np.argsort(k2s, kind="stable")
    cnt2 = np.bincount(k2s, minlength=c.NC * c.NW * c.NCH)
    st2 = np.zeros(len(cnt2) + 1, dtype=np.int64)
    np.cumsum(cnt2, out=st2[1:])
    sprank = np.empty(len(k2s), dtype=np.int64)
    sprank[o2] = np.arange(len(k2s)) - st2[k2s[o2]]
    slot[sp] = (cell_slot0[w_of[sp], ch_of[sp]] + Kwc[w_of[sp], ch_of[sp]] * 128
                + sprank)

    gidx = np.tile(gdefault, (c.NC, 1))
    dstloc = np.full((c.NC, NSPILL * 128), -1.0, dtype=np.float32)
    for core in range(c.NC):
        e = core_of == core
        gidx[core, slot[e]] = loc_of[e]
        spc = e & sp
        sloc = (slot[spc] - cell_slot0[w_of[spc], ch_of[spc]]
                - Kwc[w_of[spc], ch_of[spc]] * 128)
        dstloc[core, spill_col0[w_of[spc], ch_of[spc]] * 128 + sloc] = \
            m_of[spc]
    dstloc_sb = np.ascontiguousarray(
        dstloc.reshape(c.NC, NSPILL, 128).transpose(0, 2, 1))  # [NC,128,NSPILL]

    # ---- decode: label edge j -> core owning src --------------------------
    ls = np.asarray(edge_label_index[0], dtype=np.int64)
    ld = np.asarray(edge_label_index[1], dtype=np.int64)
    dcore = ls // c.SHARD
    drow = row_of[ld]
    dch = drow // c.CHROWS
    dkey = dcore * c.NCH + dch
    dorder = np.argsort(dkey, kind="stable")
    dcounts = np.bincount(dkey, minlength=c.NC * c.NCH).reshape(c.NC, c.NCH)
    Bdec = 128 * np.maximum(1, np.ceil(dcounts.max(axis=0) / 128).astype(np.int64))
    dstarts = np.zeros(c.NC * c.NCH + 1, dtype=np.int64)
    np.cumsum(dcounts.reshape(-1), out=dstarts[1:])
    doff = np.zeros(c.NCH + 1, dtype=np.int64)
    np.cumsum(Bdec, out=doff[1:])
    TOT_DEC = int(doff[-1])
    sidx = np.zeros((c.NC, TOT_DEC), dtype=np.int64)
    didx = np.zeros((c.NC, TOT_DEC), dtype=np.int64)
    slot2j = np.full((c.NC, TOT_DEC), -1, dtype=np.int64)
    for core in range(c.NC):
        for ch in range(c.NCH):
            g = core * c.NCH + ch
            js = dorder[dstarts[g]:dstarts[g + 1]]
            n = len(js)
            p0 = doff[ch]
            sidx[core, p0:p0 + n] = perm_pos[ls[js]]
            didx[core, p0:p0 + n] = drow[js] % c.CHROWS
            slot2j[core, p0:p0 + n] = js

    # ---- tensors ----------------------------------------------------------
    xp = np.zeros((c.NP, c.D), dtype=np.float32)
    xp[:c.N] = np.asarray(x, dtype=np.float32)
    table0 = np.zeros((c.TROWS, c.D), dtype=np.float32)
    table0[row_of] = xp * dinv_p[:, None]
    table0 = table0.astype(BF)                       # replicated to all cores
    dinv_slot = np.zeros(c.NP, dtype=np.float32)     # by (core, slot p)
    dinv_slot[(g_all // c.SHARD) * c.SHARD + perm_pos] = dinv_p
    dinv_w = np.ascontiguousarray(
        dinv_slot.reshape(c.NC, c.NW, 128).transpose(0, 2, 1))  # [NC,128,NW]
    b1r = np.tile(np.asarray(b1, np.float32)[None, :], (128, 1))
    b2r = np.tile(np.asarray(b2, np.float32)[None, :], (128, 1))

    # own-shard table0 rows in SBUF layout [128, NW, D]: slot w*128+m -> [m, w, :]
    self0 = np.ascontiguousarray(
        np.asarray(table0).reshape(c.NC, c.SROWS, c.D)[:, :c.SHARD]
        .reshape(c.NC, c.NW, 128, c.D).transpose(0, 2, 1, 3)
        .reshape(c.NC, 128, c.NW * c.D))

    in_maps = []
    for core in range(c.NC):
        in_maps.append({
            "table0": table0,
            "self0": self0[core],
            "W1": np.asarray(W1, dtype=np.float32).astype(BF),
            "W2": np.asarray(W2, dtype=np.float32).astype(BF),
            "b1r": b1r, "b2r": b2r,
            "dinvw": dinv_w[core],
            "gidx": _wrap_idxs(gidx[core]),
            "dstloc": dstloc_sb[core],
            "sidx": _wrap_idxs(sidx[core]),
            "didx": _wrap_idxs(didx[core]),
        })
    meta = dict(Kwc=Kwc, SBwc=SBwc, cell_slot0=cell_slot0,
                spill_col0=spill_col0, TOT=TOT, NBLK=NBLK, NSPILL=NSPILL,
                goff=goff, Bdec=Bdec, doff=doff,
                TOT_DEC=TOT_DEC, slot2j=slot2j)
    return in_maps, meta


def build_program(cfg, meta, num_cores=None):
    c = cfg
    NCores = num_cores or c.NC
    Kwc, SBwc, TOT = meta["Kwc"], meta["SBwc"], meta["TOT"]
    cell_slot0, spill_col0 = meta["cell_slot0"], meta["spill_col0"]
    NSPILL, goff = meta["NSPILL"], meta["goff"]
    Bdec, doff, TOT_DEC = meta["Bdec"], meta["doff"], meta["TOT_DEC"]
    D = c.D
    NB_DEC = TOT_DEC // 128

    nc = bacc.Bacc("TRN2", target_bir_lowering=False, debug=False,
                   num_devices=NCores, num_swdge_queues=4)

    table0 = nc.dram_tensor("table0", [c.TROWS, D], BF16, kind="ExternalInput")
    self0_in = nc.dram_tensor("self0", [128, c.NW * D], BF16, kind="ExternalInput")
    W1_in = nc.dram_tensor("W1", [D, D], BF16, kind="ExternalInput")
    W2_in = nc.dram_tensor("W2", [D, D], BF16, kind="ExternalInput")
    b1_in = nc.dram_tensor("b1r", [128, D], F32, kind="ExternalInput")
    b2_in = nc.dram_tensor("b2r", [128, D], F32, kind="ExternalInput")
    dinvw_in = nc.dram_tensor("dinvw", [128, c.NW], F32, kind="ExternalInput")
    gidx_in = nc.dram_tensor("gidx", [128, TOT // 16], I16, kind="ExternalInput")
    dstloc_in = nc.dram_tensor("dstloc", [128, NSPILL], F32, kind="ExternalInput")
    sidx_in = nc.dram_tensor("sidx", [128, TOT_DEC // 16], I16, kind="ExternalInput")
    didx_in = nc.dram_tensor("didx", [128, TOT_DEC // 16], I16, kind="ExternalInput")
    dots_out = nc.dram_tensor("dots", [128, NB_DEC], F32, kind="ExternalOutput")

    shard1 = nc.dram_tensor("shard1", [c.SROWS, D], BF16)
    shardz = nc.dram_tensor("shardz", [c.SROWS, D], BF16)
    table1 = nc.dram_tensor("table1", [c.TROWS, D], BF16)
    tablez = nc.dram_tensor("tablez", [c.TROWS, D], BF16)

    iota_dram = nc.inline_tensor(
        np.tile(np.arange(128, dtype=np.float32), (128, 1)).astype(BF), "iota_c")
    ident_dram = nc.inline_tensor(np.eye(128, dtype=np.float32).astype(BF), "ident_c")

    cc_sem = nc.alloc_semaphore("cc_sem")
    core_ids = list(range(NCores))

    gst = {"count": 0, "qprev": {}}

    def emit_gather(out_ap, in_ap, idx_ap, n_idx):
        q = gst["count"] % 4
        inst = nc.gpsimd.dma_gather(out_ap, in_ap, idx_ap, n_idx, n_idx, D,
                                    queue_num=q, single_packet=False)
        if q in gst["qprev"]:
            add_dep_helper(inst.ins, gst["qprev"][q].ins, sync=False,
                           reason="pin swdge queue order")
        gst["qprev"][q] = inst
        gst["count"] += 1
        return inst

    with tile.TileContext(nc) as tc:
        with contextlib.ExitStack() as es:
            const = es.enter_context(tc.tile_pool(name="const", bufs=1))
            meta_p = es.enter_context(tc.tile_pool(name="meta", bufs=1))

            w1_sb = const.tile([D, D], BF16); nc.sync.dma_start(w1_sb[:], W1_in[:])
            w2_sb = const.tile([D, D], BF16); nc.sync.dma_start(w2_sb[:], W2_in[:])
            b1_sb = const.tile([128, D], F32); nc.sync.dma_start(b1_sb[:], b1_in[:])
            b2_sb = const.tile([128, D], F32); nc.sync.dma_start(b2_sb[:], b2_in[:])
            dinv_sb = const.tile([128, c.NW], F32)
            nc.sync.dma_start(dinv_sb[:], dinvw_in[:])
            iota_sb = const.tile([128, 128], BF16)
            nc.sync.dma_start(iota_sb[:], iota_dram[:])
            ident_sb = const.tile([128, 128], BF16)
            nc.sync.dma_start(ident_sb[:], ident_dram[:])
            gidx_sb = meta_p.tile([128, TOT // 16], I16)
            nc.sync.dma_start(gidx_sb[:], gidx_in[:])
            dstloc_sb = meta_p.tile([128, NSPILL], F32)
            nc.sync.dma_start(dstloc_sb[:], dstloc_in[:])
            self0_sb = meta_p.tile([128, c.NW, D], BF16)
            nc.sync.dma_start(self0_sb[:], self0_in[:])
            zero_sb = const.tile([128, D], BF16)
            nc.vector.memset(zero_sb[:], 0.0)

            def all_gather(shard, table_out, n_before):
                tc.strict_bb_all_engine_barrier()
                with tc.tile_critical():
                    nc.gpsimd.collective_compute(
                        "AllGather", mybir.AluOpType.bypass,
                        replica_groups=[core_ids],
                        ins=[shard[:]], outs=[table_out[:]],
                    ).then_inc(cc_sem)
                    nc.gpsimd.wait_ge(cc_sem, n_before + 1)
                tc.strict_bb_all_engine_barrier()

            def layer(lid, table, bias_sb, shard_next, self_src, t4_keep):
                """self_src: callable w -> rhs AP for the self-contribution
                block; t4_keep: pool to retain dinv*z tiles (layer 1) or
                None (layer 2)."""
                kept = []
                with tc.tile_pool(name=f"M{lid}", bufs=2) as Mp, \
                     tc.tile_pool(name=f"S{lid}", bufs=6) as Sp, \
                     tc.tile_pool(name=f"agg{lid}", bufs=3, space="PSUM") as agp, \
                     tc.tile_pool(name=f"mm{lid}", bufs=2, space="PSUM") as mmp, \
                     tc.tile_pool(name=f"tp{lid}", bufs=2, space="PSUM") as tpp, \
                     tc.tile_pool(name=f"ev{lid}", bufs=8) as evp:
                    for b in range(c.NBATCH):
                        blk0 = goff[(b, 0)][0] // 128
                        nblk_b = sum(goff[(b, ch)][1]
                                     for ch in range(c.NCH)) // 128
                        Mt = Mp.tile([128, nblk_b, D], BF16, tag="M")
                        with tc.high_priority():
                            for ch in range(c.NCH):
                                off, n_idx = goff[(b, ch)]
                                half = (n_idx // 256) * 128
                                for o0, nn in ((0, half), (half, n_idx - half)):
                                    if nn == 0:
                                        continue
                                    o = off + o0
                                    emit_gather(
                                        Mt[:, o // 128 - blk0:
                                           (o + nn) // 128 - blk0, :],
                                        table[ch * c.CHROWS:
                                              (ch + 1) * c.CHROWS, :],
                                        gidx_sb[:, o // 16:(o + nn) // 16],
                                        nn)
                        for w in range(b * c.WB, (b + 1) * c.WB):
                            ps = agp.tile([128, D], F32, tag="agg")
                            # self contribution, starts the accumulation
                            nc.tensor.matmul(ps[:], lhsT=ident_sb[:],
                                             rhs=self_src(w),
                                             start=True, stop=False)
                            nmm = sum(int(Kwc[w, ch] + SBwc[w, ch])
                                      for ch in range(c.NCH))
                            k = 0
                            # structured blocks: identity weights
                            for ch in range(c.NCH):
                                gblk = cell_slot0[w, ch] // 128
                                for t in range(int(Kwc[w, ch])):
                                    nc.tensor.matmul(
                                        ps[:], lhsT=ident_sb[:],
                                        rhs=Mt[:, gblk + t - blk0, :],
                                        start=False, stop=(k == nmm - 1))
                                    k += 1
                            # spill blocks: one-hot S built on DVE
                            for ch in range(c.NCH):
                                gblk = cell_slot0[w, ch] // 128 + int(Kwc[w, ch])
                                scol = int(spill_col0[w, ch])
                                for t in range(int(SBwc[w, ch])):
                                    S = Sp.tile([128, 128], BF16, tag="S")
                                    nc.vector.tensor_scalar(
                                        S[:], iota_sb[:],
                                        dstloc_sb[:, scol + t:scol + t + 1],
                                        None, mybir.AluOpType.is_equal)
                                    nc.tensor.matmul(
                                        ps[:], lhsT=S[:],
                                        rhs=Mt[:, gblk + t - blk0, :],
                                        start=False, stop=(k == nmm - 1))
                                    k += 1
                            # eviction: agg -> aggT -> @W -> *dinv+b -> relu
                            aggb = evp.tile([128, D], BF16, tag="aggb")
                            nc.scalar.activation(
                                aggb[:], ps[:], mybir.ActivationFunctionType.Copy)
                            psT = tpp.tile([128, D], BF16, tag="aggT")
                            nc.tensor.transpose(psT[:], aggb[:], ident_sb[:])
                            aggT = evp.tile([128, D], BF16, tag="aggTs")
                            nc.scalar.activation(
                                aggT[:], psT[:], mybir.ActivationFunctionType.Copy)
                            wsb = w1_sb if lid == 1 else w2_sb
                            ps2 = mmp.tile([128, D], F32, tag="mm")
                            nc.tensor.matmul(ps2[:], lhsT=aggT[:], rhs=wsb[:],
                                             start=True, stop=True)
                            sc = evp.tile([128, D], F32, tag="sc")
                            nc.vector.tensor_scalar(
                                sc[:], ps2[:], dinv_sb[:, w:w + 1], None,
                                mybir.AluOpType.mult)
                            pre = evp.tile([128, D], F32, tag="pre")
                            nc.vector.tensor_tensor(
                                pre[:], sc[:], bias_sb[:],
                                op=mybir.AluOpType.add)
                            z = evp.tile([128, D], BF16, tag="z")
                            nc.scalar.activation(
                                z[:], pre[:], mybir.ActivationFunctionType.Relu)
                            if t4_keep is not None:
                                t4 = t4_keep.tile([128, D], BF16, tag="t4")
                                nc.vector.tensor_scalar(
                                    t4[:], z[:], dinv_sb[:, w:w + 1], None,
                                    mybir.AluOpType.mult)
                                out_t = t4
                                kept.append(t4)
                            else:
                                out_t = z
                            nc.sync.dma_start(
                                shard_next[w * 128:(w + 1) * 128, :], out_t[:])
                    # zero tail rows (gather pad target after AllGather)
                    nc.sync.dma_start(
                        shard_next[c.SHARD:c.SROWS, :], zero_sb[:])
                return kept

            with tc.tile_pool(name="t4p", bufs=c.NW) as t4p:
                kept = layer(1, table0, b1_sb, shard1,
                             self_src=lambda w: self0_sb[:, w, :],
                             t4_keep=t4p)
                all_gather(shard1, table1, 0)
                layer(2, table1, b2_sb, shardz,
                      self_src=lambda w, kept=kept: kept[w][:],
                      t4_keep=None)

            # ---- decode ----------------------------------------------------
            with tc.tile_pool(name="didx", bufs=1) as didxp, \
                 tc.tile_pool(name="dM", bufs=1) as dMp, \
                 tc.tile_pool(name="dout", bufs=2) as doutp:
                ds_sb = didxp.tile([128, TOT_DEC // 16], I16)
                nc.sync.dma_start(ds_sb[:], sidx_in[:])
                dd_sb = didxp.tile([128, TOT_DEC // 16], I16)
                nc.sync.dma_start(dd_sb[:], didx_in[:])
                Ms = dMp.tile([128, NB_DEC, D], BF16, tag="Ms")
                Md = dMp.tile([128, NB_DEC, D], BF16, tag="Md")

                all_gather(shardz, tablez, 1)
                emit_gather(Ms[:, :, :], shardz[:, :], ds_sb[:, :], TOT_DEC)

                for ch in range(c.NCH):
                    off = int(doff[ch]); n_idx = int(Bdec[ch])
                    emit_gather(
                        Md[:, off // 128:(off + n_idx) // 128, :],
                        tablez[ch * c.CHROWS:(ch + 1) * c.CHROWS, :],
                        dd_sb[:, off // 16:(off + n_idx) // 16], n_idx)
                prod = doutp.tile([128, NB_DEC, D], BF16, tag="prod")
                nc.vector.tensor_tensor(prod[:], Ms[:], Md[:],
                                        op=mybir.AluOpType.mult)
                res = doutp.tile([128, NB_DEC], F32, tag="res")
                nc.vector.tensor_reduce(res[:], prod[:],
                                        axis=mybir.AxisListType.X,
                                        op=mybir.AluOpType.add)
                nc.sync.dma_start(dots_out[:], res[:])

    nc.compile()
    return nc


def assemble_output(cfg, meta, results):
    c = cfg
    slot2j = meta["slot2j"]
    out = np.zeros(c.EL, dtype=np.float32)
    for core in range(len(results)):
        d = np.asarray(results[core]["dots"], dtype=np.float32)
        flat = d.T.reshape(-1)             # slot i -> d[i%128, i//128]
        s2j = slot2j[core]
        valid = s2j >= 0
        out[s2j[valid]] = flat[valid]
    return out


def run_pipeline(x, edge_index, edge_label_index, W1, b1, W2, b2,
                 cfg=None, trace=False, tmpdir=None):
    cfg = cfg or DEFAULT
    in_maps, meta = host_prep(cfg, x, edge_index, edge_label_index,
                              W1, b1, W2, b2)
    nc = build_program(cfg, meta)
    res = run_bass_kernel_spmd(nc, in_maps, list(range(cfg.NC)),
                               trace=trace, tmpdir=tmpdir)
    return assemble_output(cfg, meta, res.results), res


def kernel(x, edge_index, edge_label_index, W1, b1, W2, b2):
    out, _ = run_pipeline(x, edge_index, edge_label_index, W1, b1, W2, b2)
    return out


# revision 23
# speedup vs baseline: 1.1352x; 1.1352x over previous
"""Trainium2 Bass kernel for nn_LinkPredictor (2-layer GCN + edge-dot decode).

Strategy (8 NeuronCores, SPMD), v2 architecture:
  - Nodes sharded: core c owns rows [c*12544, (c+1)*12544).
  - Scale folding: table rows are pre-scaled by dinv[node]; the remaining
    dinv[dst] factor is applied per-window after aggregation (it commutes
    with the right-multiplication by W).  GCN conv = relu(dinv*(agg @ W)+b),
    agg[m] = self_row[m] + sum_e table[src[e]].
  - Layer-1 table (dinv*x, bf16) is host-replicated to every core: no
    AllGather needed before layer 1.  Only 2 AllGathers total (table1, tablez).
  - Edges grouped by (dst window of 128, src chunk of 25088); slots padded
    to 128-multiples per group.  Slot -> dst position is applied via one-hot
    S tiles built on DVE with a single is_equal op (pads get dstloc=-1 so
    their S row is zero; pad gathers read row 0, finite garbage * 0 = 0).
  - Aggregation: PSUM accumulation of S_t^T @ Mt_t (PE), self term via
    identity-weights matmul of the core's own contiguous table rows.
  - Per window: transpose agg (PE), matmul aggT @ W (PE), fused
    (ps*dinv[m])+b (DVE scalar_tensor_tensor), relu (Scalar engine).
  - Decode: label edge j assigned to the core owning src; s-side gathered
    from the core's own shardz (overlaps the last AllGather), d-side from
    the all-gathered tablez; dot = tensor_tensor mult + reduce.
"""
import contextlib
import math
import numpy as np
import ml_dtypes

import concourse.bass as bass
import concourse.tile as tile
from concourse import bacc, mybir
from concourse.bass_utils import run_bass_kernel_spmd
from concourse.tile_rust import add_dep_helper

F32 = mybir.dt.float32
BF16 = mybir.dt.bfloat16
I16 = mybir.dt.int16
BF = ml_dtypes.bfloat16


class Cfg:
    def __init__(self, N=100000, E=1600000, EL=100000, D=128, ncores=8,
                 nw=98, nchunks=4, wb=7):
        self.N, self.E, self.EL, self.D, self.NC = N, E, EL, D, ncores
        self.NW = nw                      # windows (128 nodes each) per core
        self.SHARD = nw * 128             # real nodes per core (12544)
        self.SROWS = self.SHARD + 128     # shard rows incl zero tail (12672)
        self.NP = self.SHARD * ncores     # real node count padded (100352)
        self.TROWS = self.SROWS * ncores  # table rows (101376)
        assert self.NP >= N
        self.NCH = nchunks                # src chunks (int16 index range)
        self.CHROWS = self.TROWS // nchunks  # 25344
        assert self.CHROWS <= 32767
        self.ZLOC = self.SHARD            # chunk-local zero row (12544)
        self.WB = wb                      # windows per gather batch
        assert nw % wb == 0
        self.NBATCH = nw // wb


DEFAULT = Cfg()


def _wrap_idxs(idx):
    """[n] ints -> [128, n//16] int16 wrapped in 16 partitions, replicated 8x."""
    n = len(idx)
    assert n % 16 == 0
    w = np.asarray(idx, dtype=np.int16).reshape(n // 16, 16).T
    return np.ascontiguousarray(np.tile(w, (8, 1)))


def host_prep(cfg, x, edge_index, edge_label_index, W1, b1, W2, b2):
    c = cfg
    src = np.asarray(edge_index[0], dtype=np.int64)
    dst = np.asarray(edge_index[1], dtype=np.int64)
    deg = np.bincount(dst, minlength=c.N).astype(np.float64) + 1.0
    dinv = 1.0 / np.sqrt(deg)                       # [N]
    dinv_p = np.zeros(c.NP, dtype=np.float32)
    dinv_p[:c.N] = dinv.astype(np.float32)

    # ---- per-core node permutation: sort by degree (desc) -----------------
    deg_p = np.zeros(c.NP, dtype=np.int64)
    deg_p[:c.N] = (deg - 1).astype(np.int64)
    perm_pos = np.empty(c.NP, dtype=np.int64)       # local node r -> slot p
    for core in range(c.NC):
        d_loc = deg_p[core * c.SHARD:(core + 1) * c.SHARD]
        o = np.argsort(-d_loc, kind="stable")
        pp = np.empty(c.SHARD, dtype=np.int64)
        pp[o] = np.arange(c.SHARD)
        perm_pos[core * c.SHARD:(core + 1) * c.SHARD] = pp
    # global node g -> table row
    g_all = np.arange(c.NP, dtype=np.int64)
    row_of = (g_all // c.SHARD) * c.SROWS + perm_pos

    # ---- edge slot structure ---------------------------------------------
    core_of = dst // c.SHARD
    p_of = perm_pos[dst]                            # dst slot within core
    w_of = p_of // 128
    m_of = p_of % 128
    srow = row_of[src]
    ch_of = srow // c.CHROWS
    loc_of = srow % c.CHROWS

    # rank of edge within (core, w, ch, m)
    key1 = (((core_of * c.NW + w_of) * c.NCH + ch_of) * 128 + m_of)
    order = np.argsort(key1, kind="stable")
    cnt1 = np.bincount(key1, minlength=c.NC * c.NW * c.NCH * 128)
    st1 = np.zeros(len(cnt1) + 1, dtype=np.int64)
    np.cumsum(cnt1, out=st1[1:])
    rank = np.empty(c.E, dtype=np.int64)
    rank[order] = np.arange(c.E) - st1[key1[order]]

    # choose K (structured depth) and SB (spill blocks) per (w, ch)
    cnts = cnt1.reshape(c.NC, c.NW, c.NCH, 128)
    KMAX = 24
    best_cost = None
    Kwc = np.zeros((c.NW, c.NCH), dtype=np.int64)
    SBwc = np.zeros((c.NW, c.NCH), dtype=np.int64)
    for K in range(KMAX + 1):
        spill = np.maximum(cnts - K, 0).sum(axis=3)          # [NC, NW, NCH]
        SB = np.ceil(spill / 128).astype(np.int64).max(axis=0)  # [NW, NCH]
        cost = K + SB + 0.01 * SB
        if best_cost is None:
            best_cost = cost.astype(np.float64)
            Kwc[:] = K
            SBwc[:] = SB
        else:
            better = cost < best_cost
            best_cost = np.where(better, cost, best_cost)
            Kwc = np.where(better, K, Kwc)
            SBwc = np.where(better, SB, SBwc)

    # cell layout: batch -> chunk -> window; cell = K*128 + SB*128 slots
    cell_slot0 = np.zeros((c.NW, c.NCH), dtype=np.int64)
    spill_col0 = np.zeros((c.NW, c.NCH), dtype=np.int64)
    goff = {}
    pos = 0
    spill_blocks = 0
    for b in range(c.NBATCH):
        for ch in range(c.NCH):
            run0 = pos
            for w in range(b * c.WB, (b + 1) * c.WB):
                cell_slot0[w, ch] = pos
                spill_col0[w, ch] = spill_blocks
                pos += (Kwc[w, ch] + SBwc[w, ch]) * 128
                spill_blocks += SBwc[w, ch]
            goff[(b, ch)] = (run0, pos - run0)
    TOT = pos
    NBLK = TOT // 128
    NSPILL = max(spill_blocks, 1)

    # default gidx (pads): structured -> zero-tail rows (spread over all 128
    # to avoid a DRAM hot row), spill -> rows 0..127 (values killed by S)
    gdefault = np.arange(TOT, dtype=np.int64) % 128
    for w in range(c.NW):
        for ch in range(c.NCH):
            s0 = cell_slot0[w, ch]
            gdefault[s0:s0 + Kwc[w, ch] * 128] += c.ZLOC

    # per-edge slot
    K_e = Kwc[w_of, ch_of]
    is_struct = rank < K_e
    slot = np.empty(c.E, dtype=np.int64)
    slot[is_struct] = (cell_slot0[w_of[is_struct], ch_of[is_struct]]
                       + rank[is_struct] * 128 + m_of[is_struct])
    # spill rank within (core, w, ch)
    sp = ~is_struct
    key2 = (core_of * c.NW + w_of) * c.NCH + ch_of
    k2s = key2[sp]
    o2 = np.argsort(k2s, kind="stable")
    cnt2 = np.bincount(k2s, minlength=c.NC * c.NW * c.NCH)
    st2 = np.zeros(len(cnt2) + 1, dtype=np.int64)
    np.cumsum(cnt2, out=st2[1:])
    sprank = np.empty(len(k2s), dtype=np.int64)
    sprank[o2] = np.arange(len(k2s)) - st2[k2s[o2]]
    slot[sp] = (cell_slot0[w_of[sp], ch_of[sp]] + Kwc[w_of[sp], ch_of[sp]] * 128
                + sprank)

    gidx = np.tile(gdefault, (c.NC, 1))
    dstloc = np.full((c.NC, NSPILL * 128), -1.0, dtype=np.float32)
    for core in range(c.NC):
        e = core_of == core
        gidx[core, slot[e]] = loc_of[e]
        spc = e & sp
        sloc = (slot[spc] - cell_slot0[w_of[spc], ch_of[spc]]
                - Kwc[w_of[spc], ch_of[spc]] * 128)
        dstloc[core, spill_col0[w_of[spc], ch_of[spc]] * 128 + sloc] = \
            m_of[spc]
    dstloc_sb = np.ascontiguousarray(
        dstloc.reshape(c.NC, NSPILL, 128).transpose(0, 2, 1))  # [NC,128,NSPILL]

    # ---- decode: label edge j -> core owning src --------------------------
    ls = np.asarray(edge_label_index[0], dtype=np.int64)
    ld = np.asarray(edge_label_index[1], dtype=np.int64)
    dcore = ls // c.SHARD
    drow = row_of[ld]
    dch = drow // c.CHROWS
    dkey = dcore * c.NCH + dch
    dorder = np.argsort(dkey, kind="stable")
    dcounts = np.bincount(dkey, minlength=c.NC * c.NCH).reshape(c.NC, c.NCH)
    Bdec = 128 * np.maximum(1, np.ceil(dcounts.max(axis=0) / 128).astype(np.int64))
    dstarts = np.zeros(c.NC * c.NCH + 1, dtype=np.int64)
    np.cumsum(dcounts.reshape(-1), out=dstarts[1:])
    doff = np.zeros(c.NCH + 1, dtype=np.int64)
    np.cumsum(Bdec, out=doff[1:])
    TOT_DEC = int(doff[-1])
    sidx = np.zeros((c.NC, TOT_DEC), dtype=np.int64)
    didx = np.zeros((c.NC, TOT_DEC), dtype=np.int64)
    slot2j = np.full((c.NC, TOT_DEC), -1, dtype=np.int64)
    for core in range(c.NC):
        for ch in range(c.NCH):
            g = core * c.NCH + ch
            js = dorder[dstarts[g]:dstarts[g + 1]]
            n = len(js)
            p0 = doff[ch]
            sidx[core, p0:p0 + n] = perm_pos[ls[js]]
            didx[core, p0:p0 + n] = drow[js] % c.CHROWS
            slot2j[core, p0:p0 + n] = js

    # ---- tensors ----------------------------------------------------------
    xp = np.zeros((c.NP, c.D), dtype=np.float32)
    xp[:c.N] = np.asarray(x, dtype=np.float32)
    table0 = np.zeros((c.TROWS, c.D), dtype=np.float32)
    table0[row_of] = xp * dinv_p[:, None]
    table0 = table0.astype(BF)                       # replicated to all cores
    dinv_slot = np.zeros(c.NP, dtype=np.float32)     # by (core, slot p)
    dinv_slot[(g_all // c.SHARD) * c.SHARD + perm_pos] = dinv_p
    dinv_w = np.ascontiguousarray(
        dinv_slot.reshape(c.NC, c.NW, 128).transpose(0, 2, 1))  # [NC,128,NW]
    b1r = np.tile(np.asarray(b1, np.float32)[None, :], (128, 1))
    b2r = np.tile(np.asarray(b2, np.float32)[None, :], (128, 1))

    # own-shard table0 rows in SBUF layout [128, NW, D]: slot w*128+m -> [m, w, :]
    self0 = np.ascontiguousarray(
        np.asarray(table0).reshape(c.NC, c.SROWS, c.D)[:, :c.SHARD]
        .reshape(c.NC, c.NW, 128, c.D).transpose(0, 2, 1, 3)
        .reshape(c.NC, 128, c.NW * c.D))

    in_maps = []
    for core in range(c.NC):
        in_maps.append({
            "table0": table0,
            "self0": self0[core],
            "W1": np.asarray(W1, dtype=np.float32).astype(BF),
            "W2": np.asarray(W2, dtype=np.float32).astype(BF),
            "b1r": b1r, "b2r": b2r,
            "dinvw": dinv_w[core],
            "gidx": _wrap_idxs(gidx[core]),
            "dstloc": dstloc_sb[core],
            "sidx": _wrap_idxs(sidx[core]),
            "didx": _wrap_idxs(didx[core]),
        })
    meta = dict(Kwc=Kwc, SBwc=SBwc, cell_slot0=cell_slot0,
                spill_col0=spill_col0, TOT=TOT, NBLK=NBLK, NSPILL=NSPILL,
                goff=goff, Bdec=Bdec, doff=doff,
                TOT_DEC=TOT_DEC, slot2j=slot2j)
    return in_maps, meta


def build_program(cfg, meta, num_cores=None):
    c = cfg
    NCores = num_cores or c.NC
    Kwc, SBwc, TOT = meta["Kwc"], meta["SBwc"], meta["TOT"]
    cell_slot0, spill_col0 = meta["cell_slot0"], meta["spill_col0"]
    NSPILL, goff = meta["NSPILL"], meta["goff"]
    Bdec, doff, TOT_DEC = meta["Bdec"], meta["doff"], meta["TOT_DEC"]
    D = c.D
    NB_DEC = TOT_DEC // 128

    nc = bacc.Bacc("TRN2", target_bir_lowering=False, debug=False,
                   num_devices=NCores, num_swdge_queues=4)

    table0 = nc.dram_tensor("table0", [c.TROWS, D], BF16, kind="ExternalInput")
    self0_in = nc.dram_tensor("self0", [128, c.NW * D], BF16, kind="ExternalInput")
    W1_in = nc.dram_tensor("W1", [D, D], BF16, kind="ExternalInput")
    W2_in = nc.dram_tensor("W2", [D, D], BF16, kind="ExternalInput")
    b1_in = nc.dram_tensor("b1r", [128, D], F32, kind="ExternalInput")
    b2_in = nc.dram_tensor("b2r", [128, D], F32, kind="ExternalInput")
    dinvw_in = nc.dram_tensor("dinvw", [128, c.NW], F32, kind="ExternalInput")
    gidx_in = nc.dram_tensor("gidx", [128, TOT // 16], I16, kind="ExternalInput")
    dstloc_in = nc.dram_tensor("dstloc", [128, NSPILL], F32, kind="ExternalInput")
    sidx_in = nc.dram_tensor("sidx", [128, TOT_DEC // 16], I16, kind="ExternalInput")
    didx_in = nc.dram_tensor("didx", [128, TOT_DEC // 16], I16, kind="ExternalInput")
    dots_out = nc.dram_tensor("dots", [128, NB_DEC], F32, kind="ExternalOutput")

    shard1 = nc.dram_tensor("shard1", [c.SROWS, D], BF16)
    shardz = nc.dram_tensor("shardz", [c.SROWS, D], BF16)
    table1 = nc.dram_tensor("table1", [c.TROWS, D], BF16)
    tablez = nc.dram_tensor("tablez", [c.TROWS, D], BF16)

    iota_dram = nc.inline_tensor(
        np.tile(np.arange(128, dtype=np.float32), (128, 1)).astype(BF), "iota_c")
    ident_dram = nc.inline_tensor(np.eye(128, dtype=np.float32).astype(BF), "ident_c")

    cc_sem = nc.alloc_semaphore("cc_sem")
    core_ids = list(range(NCores))

    gst = {"count": 0, "qprev": {}}

    def emit_gather(out_ap, in_ap, idx_ap, n_idx):
        q = gst["count"] % 4
        inst = nc.gpsimd.dma_gather(out_ap, in_ap, idx_ap, n_idx, n_idx, D,
                                    queue_num=q, single_packet=False)
        if q in gst["qprev"]:
            add_dep_helper(inst.ins, gst["qprev"][q].ins, sync=False,
                           reason="pin swdge queue order")
        gst["qprev"][q] = inst
        gst["count"] += 1
        return inst

    with tile.TileContext(nc) as tc:
        with contextlib.ExitStack() as es:
            const = es.enter_context(tc.tile_pool(name="const", bufs=1))
            meta_p = es.enter_context(tc.tile_pool(name="meta", bufs=1))

            w1_sb = const.tile([D, D], BF16); nc.sync.dma_start(w1_sb[:], W1_in[:])
            w2_sb = const.tile([D, D], BF16); nc.sync.dma_start(w2_sb[:], W2_in[:])
            b1_sb = const.tile([128, D], F32); nc.sync.dma_start(b1_sb[:], b1_in[:])
            b2_sb = const.tile([128, D], F32); nc.sync.dma_start(b2_sb[:], b2_in[:])
            dinv_sb = const.tile([128, c.NW], F32)
            nc.sync.dma_start(dinv_sb[:], dinvw_in[:])
            iota_sb = const.tile([128, 128], BF16)
            nc.sync.dma_start(iota_sb[:], iota_dram[:])
            ident_sb = const.tile([128, 128], BF16)
            nc.sync.dma_start(ident_sb[:], ident_dram[:])
            gidx_sb = meta_p.tile([128, TOT // 16], I16)
            nc.sync.dma_start(gidx_sb[:], gidx_in[:])
            dstloc_sb = meta_p.tile([128, NSPILL], F32)
            nc.sync.dma_start(dstloc_sb[:], dstloc_in[:])
            self0_sb = meta_p.tile([128, c.NW, D], BF16)
            nc.sync.dma_start(self0_sb[:], self0_in[:])
            zero_sb = const.tile([128, D], BF16)
            nc.vector.memset(zero_sb[:], 0.0)

            def all_gather(shard, table_out, n_before):
                tc.strict_bb_all_engine_barrier()
                with tc.tile_critical():
                    nc.gpsimd.collective_compute(
                        "AllGather", mybir.AluOpType.bypass,
                        replica_groups=[core_ids],
                        ins=[shard[:]], outs=[table_out[:]],
                    ).then_inc(cc_sem)
                    nc.gpsimd.wait_ge(cc_sem, n_before + 1)
                tc.strict_bb_all_engine_barrier()

            def layer(lid, table, bias_sb, shard_next, self_src, t4_keep):
                """self_src: callable w -> rhs AP for the self-contribution
                block; t4_keep: pool to retain dinv*z tiles (layer 1) or
                None (layer 2)."""
                kept = []
                with tc.tile_pool(name=f"M{lid}", bufs=2) as Mp, \
                     tc.tile_pool(name=f"S{lid}", bufs=6) as Sp, \
                     tc.tile_pool(name=f"agg{lid}", bufs=3, space="PSUM") as agp, \
                     tc.tile_pool(name=f"mm{lid}", bufs=2, space="PSUM") as mmp, \
                     tc.tile_pool(name=f"tp{lid}", bufs=2, space="PSUM") as tpp, \
                     tc.tile_pool(name=f"ev{lid}", bufs=8) as evp:
                    for b in range(c.NBATCH):
                        blk0 = goff[(b, 0)][0] // 128
                        nblk_b = sum(goff[(b, ch)][1]
                                     for ch in range(c.NCH)) // 128
                        Mt = Mp.tile([128, nblk_b, D], BF16, tag="M")
                        with tc.high_priority():
                            for ch in range(c.NCH):
                                off, n_idx = goff[(b, ch)]
                                half = (n_idx // 256) * 128
                                for o0, nn in ((0, half), (half, n_idx - half)):
                                    if nn == 0:
                                        continue
                                    o = off + o0
                                    emit_gather(
                                        Mt[:, o // 128 - blk0:
                                           (o + nn) // 128 - blk0, :],
                                        table[ch * c.CHROWS:
                                              (ch + 1) * c.CHROWS, :],
                                        gidx_sb[:, o // 16:(o + nn) // 16],
                                        nn)
                        for w in range(b * c.WB, (b + 1) * c.WB):
                            ps = agp.tile([128, D], F32, tag="agg")
                            # self contribution, starts the accumulation
                            nc.tensor.matmul(ps[:], lhsT=ident_sb[:],
                                             rhs=self_src(w),
                                             start=True, stop=False)
                            nmm = sum(int(Kwc[w, ch] + SBwc[w, ch])
                                      for ch in range(c.NCH))
                            k = 0
                            # structured blocks: identity weights
                            for ch in range(c.NCH):
                                gblk = cell_slot0[w, ch] // 128
                                for t in range(int(Kwc[w, ch])):
                                    nc.tensor.matmul(
                                        ps[:], lhsT=ident_sb[:],
                                        rhs=Mt[:, gblk + t - blk0, :],
                                        start=False, stop=(k == nmm - 1))
                                    k += 1
                            # spill blocks: one-hot S built on DVE
                            for ch in range(c.NCH):
                                gblk = cell_slot0[w, ch] // 128 + int(Kwc[w, ch])
                                scol = int(spill_col0[w, ch])
                                for t in range(int(SBwc[w, ch])):
                                    S = Sp.tile([128, 128], BF16, tag="S")
                                    nc.vector.tensor_scalar(
                                        S[:], iota_sb[:],
                                        dstloc_sb[:, scol + t:scol + t + 1],
                                        None, mybir.AluOpType.is_equal)
                                    nc.tensor.matmul(
                                        ps[:], lhsT=S[:],
                                        rhs=Mt[:, gblk + t - blk0, :],
                                        start=False, stop=(k == nmm - 1))
                                    k += 1
                            # eviction: agg -> aggT -> @W -> *dinv+b -> relu
                            aggb = evp.tile([128, D], BF16, tag="aggb")
                            nc.scalar.activation(
                                aggb[:], ps[:], mybir.ActivationFunctionType.Copy)
                            psT = tpp.tile([128, D], BF16, tag="aggT")
                            nc.tensor.transpose(psT[:], aggb[:], ident_sb[:])
                            aggT = evp.tile([128, D], BF16, tag="aggTs")
                            nc.scalar.activation(
                                aggT[:], psT[:], mybir.ActivationFunctionType.Copy)
                            wsb = w1_sb if lid == 1 else w2_sb
                            ps2 = mmp.tile([128, D], F32, tag="mm")
                            nc.tensor.matmul(ps2[:], lhsT=aggT[:], rhs=wsb[:],
                                             start=True, stop=True)
                            sc = evp.tile([128, D], F32, tag="sc")
                            nc.vector.tensor_scalar(
                                sc[:], ps2[:], dinv_sb[:, w:w + 1], None,
                                mybir.AluOpType.mult)
                            pre = evp.tile([128, D], F32, tag="pre")
                            nc.vector.tensor_tensor(
                                pre[:], sc[:], bias_sb[:],
                                op=mybir.AluOpType.add)
                            z = evp.tile([128, D], BF16, tag="z")
                            nc.scalar.activation(
                                z[:], pre[:], mybir.ActivationFunctionType.Relu)
                            if t4_keep is not None:
                                t4 = t4_keep.tile([128, D], BF16, tag="t4")
                                nc.vector.tensor_scalar(
                                    t4[:], z[:], dinv_sb[:, w:w + 1], None,
                                    mybir.AluOpType.mult)
                                out_t = t4
                                kept.append(t4)
                            else:
                                out_t = z
                            nc.sync.dma_start(
                                shard_next[w * 128:(w + 1) * 128, :], out_t[:])
                    # zero tail rows (gather pad target after AllGather)
                    nc.sync.dma_start(
                        shard_next[c.SHARD:c.SROWS, :], zero_sb[:])
                return kept

            with tc.tile_pool(name="t4p", bufs=c.NW) as t4p:
                kept = layer(1, table0, b1_sb, shard1,
                             self_src=lambda w: self0_sb[:, w, :],
                             t4_keep=t4p)
                all_gather(shard1, table1, 0)
                layer(2, table1, b2_sb, shardz,
                      self_src=lambda w, kept=kept: kept[w][:],
                      t4_keep=None)

            # ---- decode ----------------------------------------------------
            with tc.tile_pool(name="didx", bufs=1) as didxp, \
                 tc.tile_pool(name="dM", bufs=1) as dMp, \
                 tc.tile_pool(name="dout", bufs=2) as doutp:
                ds_sb = didxp.tile([128, TOT_DEC // 16], I16)
                nc.sync.dma_start(ds_sb[:], sidx_in[:])
                dd_sb = didxp.tile([128, TOT_DEC // 16], I16)
                nc.sync.dma_start(dd_sb[:], didx_in[:])
                Ms = dMp.tile([128, NB_DEC, D], BF16, tag="Ms")
                Md = dMp.tile([128, NB_DEC, D], BF16, tag="Md")

                all_gather(shardz, tablez, 1)
                emit_gather(Ms[:, :, :], shardz[:, :], ds_sb[:, :], TOT_DEC)

                for ch in range(c.NCH):
                    off = int(doff[ch]); n_idx = int(Bdec[ch])
                    emit_gather(
                        Md[:, off // 128:(off + n_idx) // 128, :],
                        tablez[ch * c.CHROWS:(ch + 1) * c.CHROWS, :],
                        dd_sb[:, off // 16:(off + n_idx) // 16], n_idx)
                prod = doutp.tile([128, NB_DEC, D], BF16, tag="prod")
                nc.vector.tensor_tensor(prod[:], Ms[:], Md[:],
                                        op=mybir.AluOpType.mult)
                res = doutp.tile([128, NB_DEC], F32, tag="res")
                nc.vector.tensor_reduce(res[:], prod[:],
                                        axis=mybir.AxisListType.X,
                                        op=mybir.AluOpType.add)
                nc.sync.dma_start(dots_out[:], res[:])

    nc.compile()
    return nc


def assemble_output(cfg, meta, results):
    c = cfg
    slot2j = meta["slot2j"]
    out = np.zeros(c.EL, dtype=np.float32)
    for core in range(len(results)):
        d = np.asarray(results[core]["dots"], dtype=np.float32)
        flat = d.T.reshape(-1)             # slot i -> d[i%128, i//128]
        s2j = slot2j[core]
        valid = s2j >= 0
        out[s2j[valid]] = flat[valid]
    return out


def run_pipeline(x, edge_index, edge_label_index, W1, b1, W2, b2,
                 cfg=None, trace=False, tmpdir=None):
    cfg = cfg or DEFAULT
    in_maps, meta = host_prep(cfg, x, edge_index, edge_label_index,
                              W1, b1, W2, b2)
    nc = build_program(cfg, meta)
    res = run_bass_kernel_spmd(nc, in_maps, list(range(cfg.NC)),
                               trace=trace, tmpdir=tmpdir)
    return assemble_output(cfg, meta, res.results), res


def kernel(x, edge_index, edge_label_index, W1, b1, W2, b2):
    out, _ = run_pipeline(x, edge_index, edge_label_index, W1, b1, W2, b2)
    return out


# revision 26
# speedup vs baseline: 1.1571x; 1.0193x over previous
"""Trainium2 Bass kernel for nn_LinkPredictor (2-layer GCN + edge-dot decode).

Strategy (8 NeuronCores, SPMD), v2 architecture:
  - Nodes sharded: core c owns rows [c*12544, (c+1)*12544).
  - Scale folding: table rows are pre-scaled by dinv[node]; the remaining
    dinv[dst] factor is applied per-window after aggregation (it commutes
    with the right-multiplication by W).  GCN conv = relu(dinv*(agg @ W)+b),
    agg[m] = self_row[m] + sum_e table[src[e]].
  - Layer-1 table (dinv*x, bf16) is host-replicated to every core: no
    AllGather needed before layer 1.  Only 2 AllGathers total (table1, tablez).
  - Edges grouped by (dst window of 128, src chunk of 25088); slots padded
    to 128-multiples per group.  Slot -> dst position is applied via one-hot
    S tiles built on DVE with a single is_equal op (pads get dstloc=-1 so
    their S row is zero; pad gathers read row 0, finite garbage * 0 = 0).
  - Aggregation: PSUM accumulation of S_t^T @ Mt_t (PE), self term via
    identity-weights matmul of the core's own contiguous table rows.
  - Per window: transpose agg (PE), matmul aggT @ W (PE), fused
    (ps*dinv[m])+b (DVE scalar_tensor_tensor), relu (Scalar engine).
  - Decode: label edge j assigned to the core owning src; s-side gathered
    from the core's own shardz (overlaps the last AllGather), d-side from
    the all-gathered tablez; dot = tensor_tensor mult + reduce.
"""
import contextlib
import math
import numpy as np
import ml_dtypes

import concourse.bass as bass
import concourse.tile as tile
from concourse import bacc, mybir
from concourse.bass_utils import run_bass_kernel_spmd
from concourse.tile_rust import add_dep_helper

F32 = mybir.dt.float32
BF16 = mybir.dt.bfloat16
I16 = mybir.dt.int16
BF = ml_dtypes.bfloat16


class Cfg:
    def __init__(self, N=100000, E=1600000, EL=100000, D=128, ncores=8,
                 nw=98, nchunks=4, wb=7):
        self.N, self.E, self.EL, self.D, self.NC = N, E, EL, D, ncores
        self.NW = nw                      # windows (128 nodes each) per core
        self.SHARD = nw * 128             # real nodes per core (12544)
        self.ZPAD = 1024                  # zero rows per shard (pad targets)
        self.SROWS = self.SHARD + self.ZPAD   # shard rows incl zero tail
        self.NP = self.SHARD * ncores     # real node count padded (100352)
        self.TROWS = self.SROWS * ncores  # table rows (108544)
        assert self.NP >= N
        self.NCH = nchunks                # src chunks (int16 index range)
        self.CHROWS = self.TROWS // nchunks  # 27136
        assert self.CHROWS <= 32767
        self.ZLOC = self.SHARD            # chunk-local zero range start
        self.WB = wb                      # windows per gather batch
        assert nw % wb == 0
        self.NBATCH = nw // wb


DEFAULT = Cfg()


def _wrap_idxs(idx):
    """[n] ints -> [128, n//16] int16 wrapped in 16 partitions, replicated 8x."""
    n = len(idx)
    assert n % 16 == 0
    w = np.asarray(idx, dtype=np.int16).reshape(n // 16, 16).T
    return np.ascontiguousarray(np.tile(w, (8, 1)))


def host_prep(cfg, x, edge_index, edge_label_index, W1, b1, W2, b2):
    c = cfg
    src = np.asarray(edge_index[0], dtype=np.int64)
    dst = np.asarray(edge_index[1], dtype=np.int64)
    deg = np.bincount(dst, minlength=c.N).astype(np.float64) + 1.0
    dinv = 1.0 / np.sqrt(deg)                       # [N]
    dinv_p = np.zeros(c.NP, dtype=np.float32)
    dinv_p[:c.N] = dinv.astype(np.float32)

    # ---- per-core node permutation: sort by degree (desc) -----------------
    deg_p = np.zeros(c.NP, dtype=np.int64)
    deg_p[:c.N] = (deg - 1).astype(np.int64)
    perm_pos = np.empty(c.NP, dtype=np.int64)       # local node r -> slot p
    for core in range(c.NC):
        d_loc = deg_p[core * c.SHARD:(core + 1) * c.SHARD]
        o = np.argsort(-d_loc, kind="stable")
        pp = np.empty(c.SHARD, dtype=np.int64)
        pp[o] = np.arange(c.SHARD)
        perm_pos[core * c.SHARD:(core + 1) * c.SHARD] = pp
    # global node g -> table row
    g_all = np.arange(c.NP, dtype=np.int64)
    row_of = (g_all // c.SHARD) * c.SROWS + perm_pos

    # ---- edge slot structure ---------------------------------------------
    core_of = dst // c.SHARD
    p_of = perm_pos[dst]                            # dst slot within core
    w_of = p_of // 128
    m_of = p_of % 128
    srow = row_of[src]
    ch_of = srow // c.CHROWS
    loc_of = srow % c.CHROWS

    # rank of edge within (core, w, ch, m)
    key1 = (((core_of * c.NW + w_of) * c.NCH + ch_of) * 128 + m_of)
    order = np.argsort(key1, kind="stable")
    cnt1 = np.bincount(key1, minlength=c.NC * c.NW * c.NCH * 128)
    st1 = np.zeros(len(cnt1) + 1, dtype=np.int64)
    np.cumsum(cnt1, out=st1[1:])
    rank = np.empty(c.E, dtype=np.int64)
    rank[order] = np.arange(c.E) - st1[key1[order]]

    # choose K (structured depth) and SB (spill blocks) per (w, ch)
    cnts = cnt1.reshape(c.NC, c.NW, c.NCH, 128)
    KMAX = 24
    best_cost = None
    Kwc = np.zeros((c.NW, c.NCH), dtype=np.int64)
    SBwc = np.zeros((c.NW, c.NCH), dtype=np.int64)
    for K in range(KMAX + 1):
        spill = np.maximum(cnts - K, 0).sum(axis=3)          # [NC, NW, NCH]
        SB = np.ceil(spill / 128).astype(np.int64).max(axis=0)  # [NW, NCH]
        cost = K + SB + 0.01 * SB
        if best_cost is None:
            best_cost = cost.astype(np.float64)
            Kwc[:] = K
            SBwc[:] = SB
        else:
            better = cost < best_cost
            best_cost = np.where(better, cost, best_cost)
            Kwc = np.where(better, K, Kwc)
            SBwc = np.where(better, SB, SBwc)

    # cell layout: batch -> chunk -> window; cell = K*128 + SB*128 slots
    cell_slot0 = np.zeros((c.NW, c.NCH), dtype=np.int64)
    spill_col0 = np.zeros((c.NW, c.NCH), dtype=np.int64)
    goff = {}
    pos = 0
    spill_blocks = 0
    for b in range(c.NBATCH):
        for ch in range(c.NCH):
            run0 = pos
            for w in range(b * c.WB, (b + 1) * c.WB):
                cell_slot0[w, ch] = pos
                spill_col0[w, ch] = spill_blocks
                pos += (Kwc[w, ch] + SBwc[w, ch]) * 128
                spill_blocks += SBwc[w, ch]
            goff[(b, ch)] = (run0, pos - run0)
    TOT = pos
    NBLK = TOT // 128
    NSPILL = max(spill_blocks, 1)

    # default gidx (pads): structured -> zero-tail rows (spread over both
    # sub-shards' 2*ZPAD zero rows to avoid DRAM hot rows), spill -> random
    # chunk rows (their values are killed by the one-hot S)
    rng = np.random.default_rng(12345)
    zrows = np.concatenate([
        np.arange(c.ZLOC, c.ZLOC + c.ZPAD),
        np.arange(c.SROWS + c.ZLOC, c.SROWS + c.ZLOC + c.ZPAD)])
    gdefault = rng.integers(0, c.CHROWS, TOT)
    for w in range(c.NW):
        for ch in range(c.NCH):
            s0 = cell_slot0[w, ch]
            n = Kwc[w, ch] * 128
            gdefault[s0:s0 + n] = zrows[rng.integers(0, len(zrows), n)]

    # per-edge slot
    K_e = Kwc[w_of, ch_of]
    is_struct = rank < K_e
    slot = np.empty(c.E, dtype=np.int64)
    slot[is_struct] = (cell_slot0[w_of[is_struct], ch_of[is_struct]]
                       + rank[is_struct] * 128 + m_of[is_struct])
    # spill rank within (core, w, ch)
    sp = ~is_struct
    key2 = (core_of * c.NW + w_of) * c.NCH + ch_of
    k2s = key2[sp]
    o2 = np.argsort(k2s, kind="stable")
    cnt2 = np.bincount(k2s, minlength=c.NC * c.NW * c.NCH)
    st2 = np.zeros(len(cnt2) + 1, dtype=np.int64)
    np.cumsum(cnt2, out=st2[1:])
    sprank = np.empty(len(k2s), dtype=np.int64)
    sprank[o2] = np.arange(len(k2s)) - st2[k2s[o2]]
    slot[sp] = (cell_slot0[w_of[sp], ch_of[sp]] + Kwc[w_of[sp], ch_of[sp]] * 128
                + sprank)

    gidx = np.tile(gdefault, (c.NC, 1))
    dstloc = np.full((c.NC, NSPILL * 128), -1.0, dtype=np.float32)
    for core in range(c.NC):
        e = core_of == core
        gidx[core, slot[e]] = loc_of[e]
        spc = e & sp
        sloc = (slot[spc] - cell_slot0[w_of[spc], ch_of[spc]]
                - Kwc[w_of[spc], ch_of[spc]] * 128)
        dstloc[core, spill_col0[w_of[spc], ch_of[spc]] * 128 + sloc] = \
            m_of[spc]
    dstloc_sb = np.ascontiguousarray(
        dstloc.reshape(c.NC, NSPILL, 128).transpose(0, 2, 1))  # [NC,128,NSPILL]

    # ---- decode: label edge j -> core owning src --------------------------
    ls = np.asarray(edge_label_index[0], dtype=np.int64)
    ld = np.asarray(edge_label_index[1], dtype=np.int64)
    dcore = ls // c.SHARD
    drow = row_of[ld]
    dch = drow // c.CHROWS
    dkey = dcore * c.NCH + dch
    dorder = np.argsort(dkey, kind="stable")
    dcounts = np.bincount(dkey, minlength=c.NC * c.NCH).reshape(c.NC, c.NCH)
    Bdec = 128 * np.maximum(1, np.ceil(dcounts.max(axis=0) / 128).astype(np.int64))
    dstarts = np.zeros(c.NC * c.NCH + 1, dtype=np.int64)
    np.cumsum(dcounts.reshape(-1), out=dstarts[1:])
    doff = np.zeros(c.NCH + 1, dtype=np.int64)
    np.cumsum(Bdec, out=doff[1:])
    TOT_DEC = int(doff[-1])
    sidx = np.zeros((c.NC, TOT_DEC), dtype=np.int64)
    didx = np.zeros((c.NC, TOT_DEC), dtype=np.int64)
    slot2j = np.full((c.NC, TOT_DEC), -1, dtype=np.int64)
    for core in range(c.NC):
        for ch in range(c.NCH):
            g = core * c.NCH + ch
            js = dorder[dstarts[g]:dstarts[g + 1]]
            n = len(js)
            p0 = doff[ch]
            sidx[core, p0:p0 + n] = perm_pos[ls[js]]
            didx[core, p0:p0 + n] = drow[js] % c.CHROWS
            slot2j[core, p0:p0 + n] = js

    # ---- tensors ----------------------------------------------------------
    xp = np.zeros((c.NP, c.D), dtype=np.float32)
    xp[:c.N] = np.asarray(x, dtype=np.float32)
    table0 = np.zeros((c.TROWS, c.D), dtype=np.float32)
    table0[row_of] = xp * dinv_p[:, None]
    table0 = table0.astype(BF)                       # replicated to all cores
    dinv_slot = np.zeros(c.NP, dtype=np.float32)     # by (core, slot p)
    dinv_slot[(g_all // c.SHARD) * c.SHARD + perm_pos] = dinv_p
    dinv_w = np.ascontiguousarray(
        dinv_slot.reshape(c.NC, c.NW, 128).transpose(0, 2, 1))  # [NC,128,NW]
    b1r = np.tile(np.asarray(b1, np.float32)[None, :], (128, 1))
    b2r = np.tile(np.asarray(b2, np.float32)[None, :], (128, 1))

    # own-shard table0 rows in SBUF layout [128, NW, D]: slot w*128+m -> [m, w, :]
    self0 = np.ascontiguousarray(
        np.asarray(table0).reshape(c.NC, c.SROWS, c.D)[:, :c.SHARD]
        .reshape(c.NC, c.NW, 128, c.D).transpose(0, 2, 1, 3)
        .reshape(c.NC, 128, c.NW * c.D))

    in_maps = []
    for core in range(c.NC):
        in_maps.append({
            "table0": table0,
            "self0": self0[core],
            "W1": np.asarray(W1, dtype=np.float32).astype(BF),
            "W2": np.asarray(W2, dtype=np.float32).astype(BF),
            "b1r": b1r, "b2r": b2r,
            "dinvw": dinv_w[core],
            "gidx": _wrap_idxs(gidx[core]),
            "dstloc": dstloc_sb[core],
            "sidx": _wrap_idxs(sidx[core]),
            "didx": _wrap_idxs(didx[core]),
        })
    meta = dict(Kwc=Kwc, SBwc=SBwc, cell_slot0=cell_slot0,
                spill_col0=spill_col0, TOT=TOT, NBLK=NBLK, NSPILL=NSPILL,
                goff=goff, Bdec=Bdec, doff=doff,
                TOT_DEC=TOT_DEC, slot2j=slot2j)
    return in_maps, meta


def build_program(cfg, meta, num_cores=None):
    c = cfg
    NCores = num_cores or c.NC
    Kwc, SBwc, TOT = meta["Kwc"], meta["SBwc"], meta["TOT"]
    cell_slot0, spill_col0 = meta["cell_slot0"], meta["spill_col0"]
    NSPILL, goff = meta["NSPILL"], meta["goff"]
    Bdec, doff, TOT_DEC = meta["Bdec"], meta["doff"], meta["TOT_DEC"]
    D = c.D
    NB_DEC = TOT_DEC // 128

    nc = bacc.Bacc("TRN2", target_bir_lowering=False, debug=False,
                   num_devices=NCores, num_swdge_queues=4)

    table0 = nc.dram_tensor("table0", [c.TROWS, D], BF16, kind="ExternalInput")
    self0_in = nc.dram_tensor("self0", [128, c.NW * D], BF16, kind="ExternalInput")
    W1_in = nc.dram_tensor("W1", [D, D], BF16, kind="ExternalInput")
    W2_in = nc.dram_tensor("W2", [D, D], BF16, kind="ExternalInput")
    b1_in = nc.dram_tensor("b1r", [128, D], F32, kind="ExternalInput")
    b2_in = nc.dram_tensor("b2r", [128, D], F32, kind="ExternalInput")
    dinvw_in = nc.dram_tensor("dinvw", [128, c.NW], F32, kind="ExternalInput")
    gidx_in = nc.dram_tensor("gidx", [128, TOT // 16], I16, kind="ExternalInput")
    dstloc_in = nc.dram_tensor("dstloc", [128, NSPILL], F32, kind="ExternalInput")
    sidx_in = nc.dram_tensor("sidx", [128, TOT_DEC // 16], I16, kind="ExternalInput")
    didx_in = nc.dram_tensor("didx", [128, TOT_DEC // 16], I16, kind="ExternalInput")
    dots_out = nc.dram_tensor("dots", [128, NB_DEC], F32, kind="ExternalOutput")

    shard1 = nc.dram_tensor("shard1", [c.SROWS, D], BF16)
    shardz = nc.dram_tensor("shardz", [c.SROWS, D], BF16)
    table1 = nc.dram_tensor("table1", [c.TROWS, D], BF16)
    tablez = nc.dram_tensor("tablez", [c.TROWS, D], BF16)

    iota_dram = nc.inline_tensor(
        np.tile(np.arange(128, dtype=np.float32), (128, 1)).astype(BF), "iota_c")
    ident_dram = nc.inline_tensor(np.eye(128, dtype=np.float32).astype(BF), "ident_c")

    cc_sem = nc.alloc_semaphore("cc_sem")
    core_ids = list(range(NCores))

    gst = {"count": 0, "qprev": {}}

    def emit_gather(out_ap, in_ap, idx_ap, n_idx):
        q = gst["count"] % 4
        inst = nc.gpsimd.dma_gather(out_ap, in_ap, idx_ap, n_idx, n_idx, D,
                                    queue_num=q, single_packet=False)
        if q in gst["qprev"]:
            add_dep_helper(inst.ins, gst["qprev"][q].ins, sync=False,
                           reason="pin swdge queue order")
        gst["qprev"][q] = inst
        gst["count"] += 1
        return inst

    with tile.TileContext(nc) as tc:
        with contextlib.ExitStack() as es:
            const = es.enter_context(tc.tile_pool(name="const", bufs=1))
            meta_p = es.enter_context(tc.tile_pool(name="meta", bufs=1))

            w1_sb = const.tile([D, D], BF16); nc.sync.dma_start(w1_sb[:], W1_in[:])
            w2_sb = const.tile([D, D], BF16); nc.sync.dma_start(w2_sb[:], W2_in[:])
            b1_sb = const.tile([128, D], F32); nc.sync.dma_start(b1_sb[:], b1_in[:])
            b2_sb = const.tile([128, D], F32); nc.sync.dma_start(b2_sb[:], b2_in[:])
            dinv_sb = const.tile([128, c.NW], F32)
            nc.sync.dma_start(dinv_sb[:], dinvw_in[:])
            iota_sb = const.tile([128, 128], BF16)
            nc.sync.dma_start(iota_sb[:], iota_dram[:])
            ident_sb = const.tile([128, 128], BF16)
            nc.sync.dma_start(ident_sb[:], ident_dram[:])
            gidx_sb = meta_p.tile([128, TOT // 16], I16)
            nc.sync.dma_start(gidx_sb[:], gidx_in[:])
            dstloc_sb = meta_p.tile([128, NSPILL], F32)
            nc.sync.dma_start(dstloc_sb[:], dstloc_in[:])
            self0_sb = meta_p.tile([128, c.NW, D], BF16)
            nc.sync.dma_start(self0_sb[:], self0_in[:])
            zero_sb = const.tile([128, D], BF16)
            nc.vector.memset(zero_sb[:], 0.0)

            def all_gather(shard, table_out, n_before):
                tc.strict_bb_all_engine_barrier()
                with tc.tile_critical():
                    nc.gpsimd.collective_compute(
                        "AllGather", mybir.AluOpType.bypass,
                        replica_groups=[core_ids],
                        ins=[shard[:]], outs=[table_out[:]],
                    ).then_inc(cc_sem)
                    nc.gpsimd.wait_ge(cc_sem, n_before + 1)
                tc.strict_bb_all_engine_barrier()

            def layer(lid, table, bias_sb, shard_next, self_src, t4_keep):
                """self_src: callable w -> rhs AP for the self-contribution
                block; t4_keep: pool to retain dinv*z tiles (layer 1) or
                None (layer 2)."""
                kept = []
                with tc.tile_pool(name=f"M{lid}", bufs=2) as Mp, \
                     tc.tile_pool(name=f"S{lid}", bufs=6) as Sp, \
                     tc.tile_pool(name=f"agg{lid}", bufs=3, space="PSUM") as agp, \
                     tc.tile_pool(name=f"mm{lid}", bufs=2, space="PSUM") as mmp, \
                     tc.tile_pool(name=f"tp{lid}", bufs=2, space="PSUM") as tpp, \
                     tc.tile_pool(name=f"ev{lid}", bufs=8) as evp:
                    for b in range(c.NBATCH):
                        blk0 = goff[(b, 0)][0] // 128
                        nblk_b = sum(goff[(b, ch)][1]
                                     for ch in range(c.NCH)) // 128
                        Mt = Mp.tile([128, nblk_b, D], BF16, tag="M")
                        with tc.high_priority():
                            for ch in range(c.NCH):
                                off, n_idx = goff[(b, ch)]
                                half = (n_idx // 256) * 128
                                for o0, nn in ((0, half), (half, n_idx - half)):
                                    if nn == 0:
                                        continue
                                    o = off + o0
                                    emit_gather(
                                        Mt[:, o // 128 - blk0:
                                           (o + nn) // 128 - blk0, :],
                                        table[ch * c.CHROWS:
                                              (ch + 1) * c.CHROWS, :],
                                        gidx_sb[:, o // 16:(o + nn) // 16],
                                        nn)
                        for w in range(b * c.WB, (b + 1) * c.WB):
                            ps = agp.tile([128, D], F32, tag="agg")
                            # self contribution, starts the accumulation
                            nc.tensor.matmul(ps[:], lhsT=ident_sb[:],
                                             rhs=self_src(w),
                                             start=True, stop=False)
                            nmm = sum(int(Kwc[w, ch] + SBwc[w, ch])
                                      for ch in range(c.NCH))
                            k = 0
                            # structured blocks: identity weights
                            for ch in range(c.NCH):
                                gblk = cell_slot0[w, ch] // 128
                                for t in range(int(Kwc[w, ch])):
                                    nc.tensor.matmul(
                                        ps[:], lhsT=ident_sb[:],
                                        rhs=Mt[:, gblk + t - blk0, :],
                                        start=False, stop=(k == nmm - 1))
                                    k += 1
                            # spill blocks: one-hot S built on DVE
                            for ch in range(c.NCH):
                                gblk = cell_slot0[w, ch] // 128 + int(Kwc[w, ch])
                                scol = int(spill_col0[w, ch])
                                for t in range(int(SBwc[w, ch])):
                                    S = Sp.tile([128, 128], BF16, tag="S")
                                    nc.vector.tensor_scalar(
                                        S[:], iota_sb[:],
                                        dstloc_sb[:, scol + t:scol + t + 1],
                                        None, mybir.AluOpType.is_equal)
                                    nc.tensor.matmul(
                                        ps[:], lhsT=S[:],
                                        rhs=Mt[:, gblk + t - blk0, :],
                                        start=False, stop=(k == nmm - 1))
                                    k += 1
                            # eviction: agg -> aggT -> @W -> *dinv+b -> relu
                            aggb = evp.tile([128, D], BF16, tag="aggb")
                            nc.scalar.activation(
                                aggb[:], ps[:], mybir.ActivationFunctionType.Copy)
                            psT = tpp.tile([128, D], BF16, tag="aggT")
                            nc.tensor.transpose(psT[:], aggb[:], ident_sb[:])
                            aggT = evp.tile([128, D], BF16, tag="aggTs")
                            nc.scalar.activation(
                                aggT[:], psT[:], mybir.ActivationFunctionType.Copy)
                            wsb = w1_sb if lid == 1 else w2_sb
                            ps2 = mmp.tile([128, D], F32, tag="mm")
                            nc.tensor.matmul(ps2[:], lhsT=aggT[:], rhs=wsb[:],
                                             start=True, stop=True)
                            sc = evp.tile([128, D], F32, tag="sc")
                            nc.vector.tensor_scalar(
                                sc[:], ps2[:], dinv_sb[:, w:w + 1], None,
                                mybir.AluOpType.mult)
                            pre = evp.tile([128, D], F32, tag="pre")
                            nc.vector.tensor_tensor(
                                pre[:], sc[:], bias_sb[:],
                                op=mybir.AluOpType.add)
                            z = evp.tile([128, D], BF16, tag="z")
                            nc.scalar.activation(
                                z[:], pre[:], mybir.ActivationFunctionType.Relu)
                            if t4_keep is not None:
                                t4 = t4_keep.tile([128, D], BF16, tag="t4")
                                nc.vector.tensor_scalar(
                                    t4[:], z[:], dinv_sb[:, w:w + 1], None,
                                    mybir.AluOpType.mult)
                                out_t = t4
                                kept.append(t4)
                            else:
                                out_t = z
                            nc.sync.dma_start(
                                shard_next[w * 128:(w + 1) * 128, :], out_t[:])
                    # zero tail rows (gather pad target after AllGather)
                    for z0 in range(c.SHARD, c.SROWS, 128):
                        nc.sync.dma_start(
                            shard_next[z0:z0 + 128, :], zero_sb[:])
                return kept

            with tc.tile_pool(name="t4p", bufs=c.NW) as t4p:
                kept = layer(1, table0, b1_sb, shard1,
                             self_src=lambda w: self0_sb[:, w, :],
                             t4_keep=t4p)
                all_gather(shard1, table1, 0)
                layer(2, table1, b2_sb, shardz,
                      self_src=lambda w, kept=kept: kept[w][:],
                      t4_keep=None)

            # ---- decode ----------------------------------------------------
            with tc.tile_pool(name="didx", bufs=1) as didxp, \
                 tc.tile_pool(name="dM", bufs=1) as dMp, \
                 tc.tile_pool(name="dout", bufs=2) as doutp:
                ds_sb = didxp.tile([128, TOT_DEC // 16], I16)
                nc.sync.dma_start(ds_sb[:], sidx_in[:])
                dd_sb = didxp.tile([128, TOT_DEC // 16], I16)
                nc.sync.dma_start(dd_sb[:], didx_in[:])
                Ms = dMp.tile([128, NB_DEC, D], BF16, tag="Ms")
                Md = dMp.tile([128, NB_DEC, D], BF16, tag="Md")

                all_gather(shardz, tablez, 1)
                emit_gather(Ms[:, :, :], shardz[:, :], ds_sb[:, :], TOT_DEC)

                for ch in range(c.NCH):
                    off = int(doff[ch]); n_idx = int(Bdec[ch])
                    emit_gather(
                        Md[:, off // 128:(off + n_idx) // 128, :],
                        tablez[ch * c.CHROWS:(ch + 1) * c.CHROWS, :],
                        dd_sb[:, off // 16:(off + n_idx) // 16], n_idx)
                prod = doutp.tile([128, NB_DEC, D], BF16, tag="prod")
                nc.vector.tensor_tensor(prod[:], Ms[:], Md[:],
                                        op=mybir.AluOpType.mult)
                res = doutp.tile([128, NB_DEC], F32, tag="res")
                nc.vector.tensor_reduce(res[:], prod[:],
                                        axis=mybir.AxisListType.X,
                                        op=mybir.AluOpType.add)
                nc.sync.dma_start(dots_out[:], res[:])

    nc.compile()
    return nc


def assemble_output(cfg, meta, results):
    c = cfg
    slot2j = meta["slot2j"]
    out = np.zeros(c.EL, dtype=np.float32)
    for core in range(len(results)):
        d = np.asarray(results[core]["dots"], dtype=np.float32)
        flat = d.T.reshape(-1)             # slot i -> d[i%128, i//128]
        s2j = slot2j[core]
        valid = s2j >= 0
        out[s2j[valid]] = flat[valid]
    return out


def run_pipeline(x, edge_index, edge_label_index, W1, b1, W2, b2,
                 cfg=None, trace=False, tmpdir=None):
    cfg = cfg or DEFAULT
    in_maps, meta = host_prep(cfg, x, edge_index, edge_label_index,
                              W1, b1, W2, b2)
    nc = build_program(cfg, meta)
    res = run_bass_kernel_spmd(nc, in_maps, list(range(cfg.NC)),
                               trace=trace, tmpdir=tmpdir)
    return assemble_output(cfg, meta, res.results), res


def kernel(x, edge_index, edge_label_index, W1, b1, W2, b2):
    out, _ = run_pipeline(x, edge_index, edge_label_index, W1, b1, W2, b2)
    return out
